# revision 22
# baseline (speedup 1.0000x reference)
"""Trainium2 Bass kernel for nn_CombinedLoss (robot trajectory + phase loss).

Strategy: pure data-parallel over batch (32 batches/core x 8 cores), bf16
inputs (tolerance is 2e-2), with the big quadratic reductions done on the
otherwise-idle PE via block-diagonal PSUM accumulation:
  MSE   = Sxx - 2*Sxg + Sgg          (exact, no boundary terms)
  vel   = Svv (Act square-accum of v) with flat-boundary host corrections
  acc   = 2*Svv - edges - 2*Scross   (Scross on PE)
  x_gt  = Sx0 + sum d10*g1 + sum d21*g2   (x0/j1/j2 sums also on PE)
Speed/phase elementwise work is balanced across Act/DVE/Pool so every
engine sits near the bf16 DMA roofline (~41us/core).
"""
import sys, os

for _p in (os.path.expanduser("~/.axon_site/_ro/trn_rl_repo"), "/opt/trn_rl_repo"):
    if os.path.isdir(_p) and _p not in sys.path:
        sys.path.insert(0, _p)

import numpy as np
import ml_dtypes
import concourse.bass as bass
import concourse.tile as tile
from concourse import bacc, mybir, bass_utils
from concourse.alu_op_type import AluOpType as OP

F32 = mybir.dt.float32
BF16 = mybir.dt.bfloat16
AF = mybir.ActivationFunctionType
AX = mybir.AxisListType
BF = ml_dtypes.bfloat16

# ---- problem constants (hardcoded) ----
B, T, D = 256, 8192, 12
NCORES = 8
BC = B // NCORES              # 32 batches per core
N = BC * T                    # 262144 frames per core
MAX_SPEED = 10.0

# robot chunking
F_R = 128                     # frames per partition row
CF_R = 128 * F_R              # 16384 frames per chunk
NCH_R = N // CF_R             # 16 chunks
WR = F_R * D                  # 1536
WE = WR + 2 * D               # 1560 (two overlap frames)
WV = WR + D                   # 1548 (129 vels)
WXG = WE + WR                 # 3096 combined x|g row

# phase chunking
F_P = 256
CF_P = 128 * F_P              # 32768
NCH_P = N // CF_P             # 8 chunks
WP = F_P                      # 256
WPE = WP + 1                  # 257

# strip columns (f32 partials, [128, NCOLS])
SVV = 0                       # 16 cols: per-chunk sum v^2
SLSE = 16
SXG1 = 17
SXG2 = 18
SCNT = 19                     # 2 cols (halves)
SCO = 21                      # 2 cols
SPD = 23                      # 4 cols: speed pen^2 quarters
DXX = 27
DXG = 28
DCR = 29
DGG = 30
SX0 = 31                      # single col, partition 0 only
NCOLS = 32


def _pin_act_tables(arch):
    """Restrict the act-table sets bass may choose so the whole kernel uses
    natural_log_exp_and_others (exp/ln/square/copy) plus one sqrt set: 2 loads
    total. Only removes choices; never claims a set holds a func it lacks."""
    from concourse.hw_specs import get_activation_tables
    t = get_activation_tables(arch)
    keep = {
        "natural_log_exp_and_others": {"Exp", "Ln", "Square", "Copy",
                                       "Identity", "Relu"},
        "sqrt_and_others": {"Sqrt", "Square", "Copy", "Identity", "Relu"},
    }
    for name, funcs in t.items():
        if name in keep:
            funcs.intersection_update({f for f in funcs if f.name in keep[name]})
        else:
            funcs.clear()


def build():
    nc = bacc.Bacc("TRN2", target_bir_lowering=False, debug=False)
    _pin_act_tables(nc.m.arch)

    xg = nc.dram_tensor("xg", [NCH_R * 128 * WXG], BF16, kind="ExternalInput")
    ph = nc.dram_tensor("ph", [4 * (N + 2)], BF16, kind="ExternalInput")
    out = nc.dram_tensor("partials", [128, NCOLS], F32, kind="ExternalOutput").ap()

    with tile.TileContext(nc) as tc:
        with tc.tile_pool(name="hold", bufs=1) as hold, \
             tc.tile_pool(name="psum", bufs=1, space="PSUM") as psp, \
             tc.tile_pool(name="tail", bufs=1) as tp, \
             tc.tile_pool(name="speed", bufs=2) as spp:
            strip = hold.tile([128, NCOLS], F32)
            s2hold = hold.tile([128, NCH_R * 512], BF16)   # speed^2, SoA groups
            seh = hold.tile([128, NCH_P * WP], BF16)
            mh = hold.tile([128, NCH_P * WPE], BF16)
            idxh = hold.tile([128, NCH_P * WPE], BF16)
            d10h = hold.tile([128, NCH_P * WP], BF16)
            d21h = hold.tile([128, NCH_P * WP], BF16)
            g1h = hold.tile([128, NCH_P * WP], BF16)
            g2h = hold.tile([128, NCH_P * WP], BF16)
            ones = hold.tile([128, 128], BF16)
            ident = hold.tile([128, 128], BF16)
            P0 = psp.tile([128, 512], F32)
            P1 = psp.tile([128, 512], F32)
            P2 = psp.tile([128, 512], F32)

            nc.gpsimd.memset(strip[:], 0.0)
            nc.gpsimd.memset(ones[:], 1.0)
            # ident[p, j] = 1 if j == p else 0
            nc.gpsimd.affine_select(ident[:], ones[:], [[1, 128]], OP.is_equal,
                                    0.0, base=0, channel_multiplier=-1)

            idr = idxh[:].rearrange("p (c j) -> p c j", j=WPE)
            mr = mh[:].rearrange("p (c j) -> p c j", j=WPE)

            def phase_tail_half(h):
                """coherence mask/co for phase chunks [4h, 4h+4)."""
                W4 = 4 * WP
                ddh = tp.tile([128, W4], BF16, tag="ph_dd")
                ddr = ddh[:].rearrange("p (c j) -> p c j", j=WP)
                nc.gpsimd.tensor_tensor(ddr, idr[:, 4 * h:4 * h + 4, 1:WPE],
                                        idr[:, 4 * h:4 * h + 4, 0:WP],
                                        OP.subtract)
                fh = tp.tile([128, W4], BF16, tag="ph_f")
                nc.vector.scalar_tensor_tensor(
                    out=fh[:], in0=ddh[:], scalar=-1.0, in1=ddh[:],
                    op0=OP.add, op1=OP.mult)
                maskh = tp.tile([128, W4], BF16, tag="ph_m")
                nc.vector.tensor_scalar(out=maskh[:], in0=fh[:], scalar1=1.0,
                                        scalar2=0.0, op0=OP.min, op1=OP.add,
                                        accum_out=strip[:, SCNT + h:SCNT + h + 1])
                msqh = tp.tile([128, W4], BF16, tag="ph_q")
                msqr = msqh[:].rearrange("p (c j) -> p c j", j=WP)
                nc.vector.tensor_tensor(msqr, mr[:, 4 * h:4 * h + 4, 1:WPE],
                                        mr[:, 4 * h:4 * h + 4, 1:WPE], OP.mult)
                coj = tp.tile([128, W4], BF16, tag="ph_c")
                nc.vector.scalar_tensor_tensor(
                    out=coj[:], in0=maskh[:], scalar=0.0, in1=msqh[:],
                    op0=OP.add, op1=OP.mult,
                    accum_out=strip[:, SCO + h:SCO + h + 1])

            with tc.tile_pool(name="robot", bufs=5) as rp, \
                 tc.tile_pool(name="phase", bufs=3) as pp:
                for c in range(NCH_R):
                    xgt = rp.tile([128, WXG], BF16)
                    nc.sync.dma_start(xgt[:], bass.AP(xg, c * 128 * WXG,
                                                      [[WXG, 128], [1, WXG]]))
                    xa = xgt[:]
                    pstr = xa.ap[0][0]
                    # v = x[n+1] - x[n]  (129 vels per row, AoS)
                    v = rp.tile([128, WV], BF16)
                    nc.vector.tensor_tensor(v[:], xgt[:, D:WV + D], xgt[:, 0:WV],
                                            OP.subtract)
                    va = v[:]
                    vstr = va.ap[0][0]
                    first = (c == 0)
                    last = (c == NCH_R - 1)
                    # PE: per 128-block k: P0[:,0:256] += [x_k|g_k]^T pair
                    #     P0[:,256:384] += v_k^T v_{k+12};  P0[:,384:512] += g_k^T g_k
                    for k in range(12):
                        kb = 128 * k
                        pair = bass.AP(xgt.tensor, xa.offset + kb,
                                       [[pstr, 128], [WE, 2], [1, 128]])
                        nc.tensor.matmul(P0[:, 0:256], xgt[:, kb:kb + 128], pair,
                                         start=(first and k == 0),
                                         stop=(last and k == 11))
                        vmov = bass.AP(v.tensor, va.offset + kb + D,
                                       [[vstr, 128], [1, 128]])
                        nc.tensor.matmul(P0[:, 256:384], v[:, kb:kb + 128], vmov,
                                         start=(first and k == 0),
                                         stop=(last and k == 11))
                        gsl = xgt[:, WE + kb:WE + kb + 128]
                        nc.tensor.matmul(P0[:, 384:512], gsl, gsl,
                                         start=(first and k == 0),
                                         stop=(last and k == 11))
                    # V2 = v^2 in SoA plane-major layout, accum -> Svv
                    V2 = rp.tile([128, WR], BF16)
                    v2ap = bass.AP(V2.tensor, V2[:].offset,
                                   [[V2[:].ap[0][0], 128], [1, F_R], [F_R, D]])
                    if c < 2:
                        nc.vector.scalar_tensor_tensor(
                            out=v2ap, in0=v[:, 0:WR], scalar=0.0,
                            in1=v[:, 0:WR], op0=OP.add, op1=OP.mult,
                            accum_out=strip[:, SVV + c:SVV + c + 1])
                    else:
                        nc.scalar.activation(v2ap, v[:, 0:WR], AF.Square,
                                             accum_out=strip[:, SVV + c:SVV + c + 1])
                    # s2 = per-(frame, group-of-3) sums from V2 planes
                    V2r = V2[:].rearrange("p (g c j) -> p g c j", c=3, j=F_R)
                    s2sl = s2hold[:, c * 512:(c + 1) * 512]
                    s2r = s2sl.rearrange("p (g j) -> p g j", j=F_R)
                    s2a = rp.tile([128, 512], BF16)
                    s2ar = s2a[:].rearrange("p (g j) -> p g j", j=F_R)
                    s2eng = nc.vector if c % 2 == 0 else nc.gpsimd
                    s2eng.tensor_tensor(s2ar, V2r[:, :, 0, :], V2r[:, :, 1, :],
                                        OP.add)
                    s2eng.tensor_tensor(s2r, s2ar, V2r[:, :, 2, :], OP.add)

                    # ---------------- phase chunk (even c) ----------------
                    if c % 2 == 0:
                        j = c // 2
                        base = j * CF_P
                        pt = pp.tile([128, 4 * WPE], BF16)
                        nc.scalar.dma_start(pt[:], bass.AP(ph, base,
                                            [[WP, 128], [N + 2, 4], [1, WPE]]))
                        x0t = pt[:, 0:WPE]
                        x1t = pt[:, WPE:2 * WPE]
                        x2t = pt[:, 2 * WPE:3 * WPE]
                        gtt = pt[:, 3 * WPE:3 * WPE + WP]
                        x0s = pt[:, 0:WP]
                        x1s = pt[:, WPE:WPE + WP]
                        x2s = pt[:, 2 * WPE:2 * WPE + WP]
                        # exp of all three planes in one op
                        et = pp.tile([128, 3 * WPE], BF16)
                        nc.scalar.activation(et[:], pt[:, 0:3 * WPE], AF.Exp)
                        sesl = seh[:, j * WP:(j + 1) * WP]
                        sea = pp.tile([128, WP], BF16)
                        nc.gpsimd.tensor_tensor(sea[:], et[:, 0:WP],
                                                et[:, WPE:WPE + WP], OP.add)
                        nc.gpsimd.tensor_tensor(sesl, sea[:],
                                                et[:, 2 * WPE:2 * WPE + WP],
                                                OP.add)
                        msl = mh[:, j * WPE:(j + 1) * WPE]
                        ma = pp.tile([128, WPE], BF16)
                        nc.vector.tensor_tensor(ma[:], x0t, x1t, OP.max)
                        nc.vector.tensor_tensor(msl, ma[:], x2t, OP.max)
                        nc.gpsimd.tensor_tensor(d10h[:, j * WP:(j + 1) * WP],
                                                x1s, x0s, OP.subtract)
                        nc.gpsimd.tensor_tensor(d21h[:, j * WP:(j + 1) * WP],
                                                x2s, x1s, OP.subtract)
                        eq1 = pp.tile([128, WPE], BF16)
                        eq2 = pp.tile([128, WPE], BF16)
                        nc.vector.tensor_tensor(eq1[:], x1t, msl, OP.is_equal)
                        nc.vector.tensor_tensor(eq2[:], x2t, msl, OP.is_equal)
                        nc.vector.scalar_tensor_tensor(
                            out=idxh[:, j * WPE:(j + 1) * WPE], in0=eq2[:],
                            scalar=2.0, in1=eq1[:], op0=OP.mult, op1=OP.add)
                        nc.vector.tensor_scalar(out=g1h[:, j * WP:(j + 1) * WP],
                                                in0=gtt, scalar1=1.0, scalar2=0.0,
                                                op0=OP.min, op1=OP.add)
                        nc.vector.tensor_scalar(out=g2h[:, j * WP:(j + 1) * WP],
                                                in0=gtt, scalar1=-1.0, scalar2=0.0,
                                                op0=OP.add, op1=OP.max)
                        nc.tensor.matmul(P1[0:1, 256:512], ones[:, 0:1],
                                         pt[:, 0:WP],
                                         start=(j == 0), stop=(j == 7))

                    if c == 9:
                        phase_tail_half(0)

                # ---------------- tail ----------------
                for j in range(NCH_P):
                    for k2 in range(2):
                        co = j * WP + 128 * k2
                        nc.tensor.matmul(P1[:, 0:128], d10h[:, co:co + 128],
                                         g1h[:, co:co + 128],
                                         start=(j == 0 and k2 == 0),
                                         stop=(j == 7 and k2 == 1))
                        nc.tensor.matmul(P1[:, 128:256], d21h[:, co:co + 128],
                                         g2h[:, co:co + 128],
                                         start=(j == 0 and k2 == 0),
                                         stop=(j == 7 and k2 == 1))
                phase_tail_half(1)
                # ln over softmax denominators (one table switch)
                lnj = tp.tile([128, NCH_P * WP], BF16, tag="pjunk")
                nc.scalar.activation(lnj[:], seh[:], AF.Ln,
                                     accum_out=strip[:, SLSE:SLSE + 1])
                # speed: r=sqrt(s2)=exp(0.5*ln(s2)) (stays in the exp/ln act
                # table set -> no table reloads), pen=max(r-10,0), sum pen^2
                for s in range(4):
                    sl = s2hold[:, s * 2048:(s + 1) * 2048]
                    lt = spp.tile([128, 2048], F32, tag="sp_l")
                    nc.scalar.activation(lt[:], sl, AF.Ln)
                    rt = spp.tile([128, 2048], BF16, tag="sp_r")
                    nc.scalar.activation(rt[:], lt[:], AF.Exp, scale=0.5)
                    pent = spp.tile([128, 2048], BF16, tag="sp_p")
                    nc.vector.tensor_scalar(out=pent[:], in0=rt[:],
                                            scalar1=-MAX_SPEED, scalar2=0.0,
                                            op0=OP.add, op1=OP.max)
                    pq = spp.tile([128, 2048], BF16, tag="sp_j")
                    nc.vector.tensor_tensor(pq[:], pent[:], pent[:], OP.mult)
                    for m in range(4):
                        nc.tensor.matmul(P2[0:1, 0:512], ones[:, 0:1],
                                         pq[:, m * 512:(m + 1) * 512],
                                         start=(s == 0 and m == 0),
                                         stop=(s == 3 and m == 3))
                # PE diag extracts
                ej = tp.tile([128, 128], F32, tag="ex")
                for i, col in enumerate((DXX, DXG, DCR, DGG)):
                    nc.vector.scalar_tensor_tensor(
                        out=ej[:], in0=P0[:, i * 128:(i + 1) * 128],
                        scalar=0.0, in1=ident[:], op0=OP.add, op1=OP.mult,
                        accum_out=strip[:, col:col + 1])
                ej2 = tp.tile([128, 128], F32, tag="ex2")
                for i, col in enumerate((SXG1, SXG2)):
                    nc.vector.scalar_tensor_tensor(
                        out=ej2[:], in0=P1[:, i * 128:(i + 1) * 128],
                        scalar=0.0, in1=ident[:], op0=OP.add, op1=OP.mult,
                        accum_out=strip[:, col:col + 1])
                nc.vector.reduce_sum(strip[0:1, SX0:SX0 + 1], P1[0:1, 256:512],
                                     axis=AX.X)
                nc.vector.reduce_sum(strip[0:1, SPD:SPD + 1], P2[0:1, 0:512],
                                     axis=AX.X)

            nc.sync.dma_start(out, strip[:])
    nc.compile()
    return nc


_NC_CACHE = None


def _get_nc():
    global _NC_CACHE
    if _NC_CACHE is None:
        _NC_CACHE = build()
    return _NC_CACHE


def _prep_core(xs, ps, gs, ts):
    """Per-core input map. xs,gs: [BC,T,D] f32; ps: [BC,T,3] f32; ts: [BC,T] i32."""
    xflat = np.zeros(((N + 2) * D,), np.float32)
    xflat[:N * D] = xs.reshape(-1)
    xw = np.lib.stride_tricks.sliding_window_view(xflat, WE)[::WR][:NCH_R * 128]
    xgrow = np.empty((NCH_R * 128, WXG), BF)
    xgrow[:, 0:WE] = xw.astype(BF)
    xgrow[:, WE:WXG] = gs.reshape(NCH_R * 128, WR).astype(BF)
    phf = np.zeros((4, N + 2), np.float32)
    phf[0:3, :N] = ps.reshape(N, 3).T
    phf[3, :N] = ts.reshape(-1)
    return {"xg": xgrow.reshape(-1), "ph": phf.astype(BF).reshape(-1)}


def _host_finish(strips, pred_robot, pred_phase):
    """strips: list of [128, NCOLS] per core. Returns f32 scalar total loss."""
    S = np.stack([s.astype(np.float64).sum(axis=0) for s in strips])  # [8, NCOLS]
    tot = S.sum(axis=0)
    svv = tot[SVV:SVV + NCH_R].sum()
    sgg = tot[DGG]
    slse = tot[SLSE]
    sxg1 = tot[SXG1]
    sxg2 = tot[SXG2]
    scnt = tot[SCNT] + tot[SCNT + 1]
    sco = tot[SCO] + tot[SCO + 1]
    sspeed = tot[SPD:SPD + 4].sum()
    sxx = tot[DXX]
    sxg = tot[DXG]
    scross = tot[DCR]
    sx0 = sum(float(s[0, SX0]) for s in strips)

    mse_sum = sxx - 2.0 * sxg + sgg

    pr = pred_robot.astype(BF).astype(np.float64)
    pp_ = pred_phase.astype(BF).astype(np.float64)

    # ---- boundary corrections (f64, tiny) ----
    svv_c = 0.0; sspeed_c = 0.0; cross_c = 0.0; edge_sum = 0.0
    cnt_c = 0.0; co_c = 0.0
    for ci in range(NCORES):
        Xb = pr[ci * BC:(ci + 1) * BC]                # [BC,T,D]
        # invalid flat vels at n = k*T-1, k=1..BC
        vbad = np.empty((BC, D))
        vbad[:BC - 1] = Xb[1:, 0] - Xb[:-1, T - 1]
        vbad[BC - 1] = -Xb[BC - 1, T - 1]             # pad-zero edge
        svv_c += (vbad ** 2).sum()
        s2b = (vbad.reshape(BC, 4, 3) ** 2).sum(-1)
        pen = np.maximum(np.sqrt(s2b) - MAX_SPEED, 0.0)
        sspeed_c += (pen ** 2).sum()
        # invalid cross products: v_{nk-1}*vbad + vbad*v_{nk+1}
        vprev = Xb[:, T - 1] - Xb[:, T - 2]           # [BC,D] last valid vel
        vnext = Xb[:, 1] - Xb[:, 0]                   # first valid vel
        cross_c += (vprev * vbad).sum()
        cross_c += (vbad[:BC - 1] * vnext[1:]).sum()
        # per-batch edge vels for the acc identity
        edge_sum += (vnext ** 2).sum() + (vprev ** 2).sum()
        # phase coherence corrections at pair t = k*T-1
        Pb = pp_[ci * BC:(ci + 1) * BC]               # [BC,T,3]
        a = Pb[:, T - 1]                              # logits at t
        b = np.zeros_like(a)
        b[:BC - 1] = Pb[1:, 0]                        # logits at t+1 (pad zero last)
        ma_ = a.max(-1); mb_ = b.max(-1)
        ua = (a[:, 1] == ma_) + 2.0 * (a[:, 2] == ma_)
        ub = (b[:, 1] == mb_) + 2.0 * (b[:, 2] == mb_)
        dd = ub - ua
        f = (dd - 1.0) * dd
        mask = np.minimum(f, 1.0)
        cnt_c += mask.sum()
        co_c += (mask * mb_ ** 2).sum()

    svv_t = svv - svv_c
    cross_t = scross - cross_c
    sspeed_t = sspeed - sspeed_c
    acc_sum = 2.0 * svv_t - edge_sum - 2.0 * cross_t
    cnt_t = scnt - cnt_c
    co_t = sco - co_c

    robot_loss = mse_sum / (B * T * D)
    xgt = sx0 + sxg1 + sxg2
    phase_loss = (slse - xgt) / (B * T)
    coherence = (co_t / max(cnt_t, 1.0)) if cnt_t > 0 else 0.0
    speed_loss = 5.0 * sspeed_t / (B * (T - 1) * 4)
    vel_loss = svv_t / (B * (T - 1) * D)
    acc_loss = acc_sum / (B * (T - 2) * D)
    total = (robot_loss + phase_loss + 10.0 * coherence + speed_loss
             + 0.05 * vel_loss + 0.01 * acc_loss)
    return np.asarray(total, dtype=np.float32)


def kernel(pred_robot, pred_phase, gt_robot, gt_phase):
    nc = _get_nc()
    in_maps = []
    for c in range(NCORES):
        sl = slice(c * BC, (c + 1) * BC)
        in_maps.append(_prep_core(pred_robot[sl], pred_phase[sl],
                                  gt_robot[sl], gt_phase[sl]))
    res = bass_utils.run_bass_kernel_spmd(nc, in_maps, core_ids=list(range(NCORES)))
    strips = [res.results[c]["partials"] for c in range(NCORES)]
    return _host_finish(strips, pred_robot, pred_phase)


# revision 23
# speedup vs baseline: 1.0192x; 1.0192x over previous
"""Trainium2 Bass kernel for nn_CombinedLoss (robot trajectory + phase loss).

Strategy: pure data-parallel over batch (32 batches/core x 8 cores), bf16
inputs (tolerance is 2e-2), with the big quadratic reductions done on the
otherwise-idle PE via block-diagonal PSUM accumulation:
  MSE   = Sxx - 2*Sxg + Sgg          (exact, no boundary terms)
  vel   = Svv (Act square-accum of v) with flat-boundary host corrections
  acc   = 2*Svv - edges - 2*Scross   (Scross on PE)
  x_gt  = Sx0 + sum d10*g1 + sum d21*g2   (x0/j1/j2 sums also on PE)
Speed/phase elementwise work is balanced across Act/DVE/Pool so every
engine sits near the bf16 DMA roofline (~41us/core).
"""
import sys, os

for _p in (os.path.expanduser("~/.axon_site/_ro/trn_rl_repo"), "/opt/trn_rl_repo"):
    if os.path.isdir(_p) and _p not in sys.path:
        sys.path.insert(0, _p)

import numpy as np
import ml_dtypes
import concourse.bass as bass
import concourse.tile as tile
from concourse import bacc, mybir, bass_utils
from concourse.alu_op_type import AluOpType as OP

F32 = mybir.dt.float32
BF16 = mybir.dt.bfloat16
AF = mybir.ActivationFunctionType
AX = mybir.AxisListType
BF = ml_dtypes.bfloat16

# ---- problem constants (hardcoded) ----
B, T, D = 256, 8192, 12
NCORES = 8
BC = B // NCORES              # 32 batches per core
N = BC * T                    # 262144 frames per core
MAX_SPEED = 10.0

# robot chunking
F_R = 128                     # frames per partition row
CF_R = 128 * F_R              # 16384 frames per chunk
NCH_R = N // CF_R             # 16 chunks
WR = F_R * D                  # 1536
WE = WR + 2 * D               # 1560 (two overlap frames)
WV = WR + D                   # 1548 (129 vels)
WXG = WE + WR                 # 3096 combined x|g row

# phase chunking
F_P = 256
CF_P = 128 * F_P              # 32768
NCH_P = N // CF_P             # 8 chunks
WP = F_P                      # 256
WPE = WP + 1                  # 257

# strip columns (f32 partials, [128, NCOLS])
SVV = 0                       # 16 cols: per-chunk sum v^2
SLSE = 16
SXG1 = 17
SXG2 = 18
SCNT = 19                     # 2 cols (halves)
SCO = 21                      # 2 cols
SPD = 23                      # 4 cols: speed pen^2 quarters
DXX = 27
DXG = 28
DCR = 29
DGG = 30
SX0 = 31                      # single col, partition 0 only
NCOLS = 32


def _pin_act_tables(arch):
    """Restrict the act-table sets bass may choose so the whole kernel uses
    natural_log_exp_and_others (exp/ln/square/copy) plus one sqrt set: 2 loads
    total. Only removes choices; never claims a set holds a func it lacks."""
    from concourse.hw_specs import get_activation_tables
    t = get_activation_tables(arch)
    keep = {
        "natural_log_exp_and_others": {"Exp", "Ln", "Square", "Copy",
                                       "Identity", "Relu"},
        "sqrt_and_others": {"Sqrt", "Square", "Copy", "Identity", "Relu"},
    }
    for name, funcs in t.items():
        if name in keep:
            funcs.intersection_update({f for f in funcs if f.name in keep[name]})
        else:
            funcs.clear()


def build():
    nc = bacc.Bacc("TRN2", target_bir_lowering=False, debug=False)
    _pin_act_tables(nc.m.arch)

    xg = nc.dram_tensor("xg", [NCH_R * 128 * WXG], BF16, kind="ExternalInput")
    ph = nc.dram_tensor("ph", [4 * (N + 2)], BF16, kind="ExternalInput")
    out = nc.dram_tensor("partials", [128, NCOLS], F32, kind="ExternalOutput").ap()

    with tile.TileContext(nc) as tc:
        with tc.tile_pool(name="hold", bufs=1) as hold, \
             tc.tile_pool(name="psum", bufs=1, space="PSUM") as psp, \
             tc.tile_pool(name="tail", bufs=1) as tp, \
             tc.tile_pool(name="speed", bufs=2) as spp:
            strip = hold.tile([128, NCOLS], F32)
            s2hold = hold.tile([128, NCH_R * 512], BF16)   # speed^2, SoA groups
            seh = hold.tile([128, NCH_P * WP], BF16)
            mh = hold.tile([128, NCH_P * WPE], BF16)
            idxh = hold.tile([128, NCH_P * WPE], BF16)
            d10h = hold.tile([128, NCH_P * WP], BF16)
            d21h = hold.tile([128, NCH_P * WP], BF16)
            g1h = hold.tile([128, NCH_P * WP], BF16)
            g2h = hold.tile([128, NCH_P * WP], BF16)
            ones = hold.tile([128, 128], BF16)
            ident = hold.tile([128, 128], BF16)
            P0 = psp.tile([128, 512], F32)
            P1 = psp.tile([128, 512], F32)

            nc.gpsimd.memset(strip[:], 0.0)
            nc.gpsimd.memset(ones[:], 1.0)
            # ident[p, j] = 1 if j == p else 0
            nc.gpsimd.affine_select(ident[:], ones[:], [[1, 128]], OP.is_equal,
                                    0.0, base=0, channel_multiplier=-1)

            idr = idxh[:].rearrange("p (c j) -> p c j", j=WPE)
            mr = mh[:].rearrange("p (c j) -> p c j", j=WPE)

            def phase_tail_half(h):
                """coherence mask/co for phase chunks [4h, 4h+4)."""
                W4 = 4 * WP
                ddh = tp.tile([128, W4], BF16, tag="ph_dd")
                ddr = ddh[:].rearrange("p (c j) -> p c j", j=WP)
                nc.gpsimd.tensor_tensor(ddr, idr[:, 4 * h:4 * h + 4, 1:WPE],
                                        idr[:, 4 * h:4 * h + 4, 0:WP],
                                        OP.subtract)
                fh = tp.tile([128, W4], BF16, tag="ph_f")
                nc.vector.scalar_tensor_tensor(
                    out=fh[:], in0=ddh[:], scalar=-1.0, in1=ddh[:],
                    op0=OP.add, op1=OP.mult)
                maskh = tp.tile([128, W4], BF16, tag="ph_m")
                nc.vector.tensor_scalar(out=maskh[:], in0=fh[:], scalar1=1.0,
                                        scalar2=0.0, op0=OP.min, op1=OP.add,
                                        accum_out=strip[:, SCNT + h:SCNT + h + 1])
                msqh = tp.tile([128, W4], BF16, tag="ph_q")
                msqr = msqh[:].rearrange("p (c j) -> p c j", j=WP)
                nc.vector.tensor_tensor(msqr, mr[:, 4 * h:4 * h + 4, 1:WPE],
                                        mr[:, 4 * h:4 * h + 4, 1:WPE], OP.mult)
                coj = tp.tile([128, W4], BF16, tag="ph_c")
                nc.vector.scalar_tensor_tensor(
                    out=coj[:], in0=maskh[:], scalar=0.0, in1=msqh[:],
                    op0=OP.add, op1=OP.mult,
                    accum_out=strip[:, SCO + h:SCO + h + 1])

            with tc.tile_pool(name="robot", bufs=5) as rp, \
                 tc.tile_pool(name="phase", bufs=3) as pp:
                for c in range(NCH_R):
                    xgt = rp.tile([128, WXG], BF16)
                    nc.sync.dma_start(xgt[:], bass.AP(xg, c * 128 * WXG,
                                                      [[WXG, 128], [1, WXG]]))
                    xa = xgt[:]
                    pstr = xa.ap[0][0]
                    # v = x[n+1] - x[n]  (129 vels per row, AoS)
                    v = rp.tile([128, WV], BF16)
                    nc.vector.tensor_tensor(v[:], xgt[:, D:WV + D], xgt[:, 0:WV],
                                            OP.subtract)
                    va = v[:]
                    vstr = va.ap[0][0]
                    first = (c == 0)
                    last = (c == NCH_R - 1)
                    # PE: per 128-block k: P0[:,0:256] += [x_k|g_k]^T pair
                    #     P0[:,256:384] += v_k^T v_{k+12};  P0[:,384:512] += g_k^T g_k
                    for k in range(12):
                        kb = 128 * k
                        pair = bass.AP(xgt.tensor, xa.offset + kb,
                                       [[pstr, 128], [WE, 2], [1, 128]])
                        nc.tensor.matmul(P0[:, 0:256], xgt[:, kb:kb + 128], pair,
                                         start=(first and k == 0),
                                         stop=(last and k == 11))
                        vmov = bass.AP(v.tensor, va.offset + kb + D,
                                       [[vstr, 128], [1, 128]])
                        nc.tensor.matmul(P0[:, 256:384], v[:, kb:kb + 128], vmov,
                                         start=(first and k == 0),
                                         stop=(last and k == 11))
                        gsl = xgt[:, WE + kb:WE + kb + 128]
                        nc.tensor.matmul(P0[:, 384:512], gsl, gsl,
                                         start=(first and k == 0),
                                         stop=(last and k == 11))
                    # V2 = v^2 in SoA plane-major layout, accum -> Svv
                    V2 = rp.tile([128, WR], BF16)
                    v2ap = bass.AP(V2.tensor, V2[:].offset,
                                   [[V2[:].ap[0][0], 128], [1, F_R], [F_R, D]])
                    if c < 2:
                        nc.vector.scalar_tensor_tensor(
                            out=v2ap, in0=v[:, 0:WR], scalar=0.0,
                            in1=v[:, 0:WR], op0=OP.add, op1=OP.mult,
                            accum_out=strip[:, SVV + c:SVV + c + 1])
                    else:
                        nc.scalar.activation(v2ap, v[:, 0:WR], AF.Square,
                                             accum_out=strip[:, SVV + c:SVV + c + 1])
                    # s2 = per-(frame, group-of-3) sums from V2 planes
                    V2r = V2[:].rearrange("p (g c j) -> p g c j", c=3, j=F_R)
                    s2sl = s2hold[:, c * 512:(c + 1) * 512]
                    s2r = s2sl.rearrange("p (g j) -> p g j", j=F_R)
                    s2a = rp.tile([128, 512], BF16)
                    s2ar = s2a[:].rearrange("p (g j) -> p g j", j=F_R)
                    s2eng = nc.vector if c % 2 == 0 else nc.gpsimd
                    s2eng.tensor_tensor(s2ar, V2r[:, :, 0, :], V2r[:, :, 1, :],
                                        OP.add)
                    s2eng.tensor_tensor(s2r, s2ar, V2r[:, :, 2, :], OP.add)

                    # ---------------- phase chunk (even c) ----------------
                    if c % 2 == 0:
                        j = c // 2
                        base = j * CF_P
                        pt = pp.tile([128, 4 * WPE], BF16)
                        nc.scalar.dma_start(pt[:], bass.AP(ph, base,
                                            [[WP, 128], [N + 2, 4], [1, WPE]]))
                        x0t = pt[:, 0:WPE]
                        x1t = pt[:, WPE:2 * WPE]
                        x2t = pt[:, 2 * WPE:3 * WPE]
                        gtt = pt[:, 3 * WPE:3 * WPE + WP]
                        x0s = pt[:, 0:WP]
                        x1s = pt[:, WPE:WPE + WP]
                        x2s = pt[:, 2 * WPE:2 * WPE + WP]
                        # exp of all three planes in one op
                        et = pp.tile([128, 3 * WPE], BF16)
                        nc.scalar.activation(et[:], pt[:, 0:3 * WPE], AF.Exp)
                        sesl = seh[:, j * WP:(j + 1) * WP]
                        sea = pp.tile([128, WP], BF16)
                        nc.gpsimd.tensor_tensor(sea[:], et[:, 0:WP],
                                                et[:, WPE:WPE + WP], OP.add)
                        nc.gpsimd.tensor_tensor(sesl, sea[:],
                                                et[:, 2 * WPE:2 * WPE + WP],
                                                OP.add)
                        msl = mh[:, j * WPE:(j + 1) * WPE]
                        ma = pp.tile([128, WPE], BF16)
                        nc.vector.tensor_tensor(ma[:], x0t, x1t, OP.max)
                        nc.vector.tensor_tensor(msl, ma[:], x2t, OP.max)
                        nc.gpsimd.tensor_tensor(d10h[:, j * WP:(j + 1) * WP],
                                                x1s, x0s, OP.subtract)
                        nc.gpsimd.tensor_tensor(d21h[:, j * WP:(j + 1) * WP],
                                                x2s, x1s, OP.subtract)
                        eq1 = pp.tile([128, WPE], BF16)
                        eq2 = pp.tile([128, WPE], BF16)
                        nc.vector.tensor_tensor(eq1[:], x1t, msl, OP.is_equal)
                        nc.vector.tensor_tensor(eq2[:], x2t, msl, OP.is_equal)
                        nc.vector.scalar_tensor_tensor(
                            out=idxh[:, j * WPE:(j + 1) * WPE], in0=eq2[:],
                            scalar=2.0, in1=eq1[:], op0=OP.mult, op1=OP.add)
                        nc.vector.tensor_scalar(out=g1h[:, j * WP:(j + 1) * WP],
                                                in0=gtt, scalar1=1.0, scalar2=0.0,
                                                op0=OP.min, op1=OP.add)
                        nc.vector.tensor_scalar(out=g2h[:, j * WP:(j + 1) * WP],
                                                in0=gtt, scalar1=-1.0, scalar2=0.0,
                                                op0=OP.add, op1=OP.max)
                        nc.tensor.matmul(P1[0:1, 256:512], ones[:, 0:1],
                                         pt[:, 0:WP],
                                         start=(j == 0), stop=(j == 7))

                    if c == 9:
                        phase_tail_half(0)

                # ---------------- tail ----------------
                for j in range(NCH_P):
                    for k2 in range(2):
                        co = j * WP + 128 * k2
                        nc.tensor.matmul(P1[:, 0:128], d10h[:, co:co + 128],
                                         g1h[:, co:co + 128],
                                         start=(j == 0 and k2 == 0),
                                         stop=(j == 7 and k2 == 1))
                        nc.tensor.matmul(P1[:, 128:256], d21h[:, co:co + 128],
                                         g2h[:, co:co + 128],
                                         start=(j == 0 and k2 == 0),
                                         stop=(j == 7 and k2 == 1))
                phase_tail_half(1)
                # ln over softmax denominators (one table switch)
                lnj = tp.tile([128, NCH_P * WP], BF16, tag="pjunk")
                nc.scalar.activation(lnj[:], seh[:], AF.Ln,
                                     accum_out=strip[:, SLSE:SLSE + 1])
                # speed: r=sqrt(s2)=exp(0.5*ln(s2)) (stays in the exp/ln act
                # table set -> no table reloads), pen=max(r-10,0), sum pen^2
                for s in range(4):
                    sl = s2hold[:, s * 2048:(s + 1) * 2048]
                    lt = spp.tile([128, 2048], F32, tag="sp_l")
                    nc.scalar.activation(lt[:], sl, AF.Ln)
                    rt = spp.tile([128, 2048], BF16, tag="sp_r")
                    nc.scalar.activation(rt[:], lt[:], AF.Exp, scale=0.5)
                    pent = spp.tile([128, 2048], BF16, tag="sp_p")
                    nc.vector.tensor_scalar(out=pent[:], in0=rt[:],
                                            scalar1=-MAX_SPEED, scalar2=0.0,
                                            op0=OP.add, op1=OP.max)
                    pj = spp.tile([128, 2048], BF16, tag="sp_j")
                    nc.vector.scalar_tensor_tensor(
                        out=pj[:], in0=pent[:], scalar=0.0, in1=pent[:],
                        op0=OP.add, op1=OP.mult,
                        accum_out=strip[:, SPD + s:SPD + s + 1])
                # PE diag extracts
                ej = tp.tile([128, 128], F32, tag="ex")
                for i, col in enumerate((DXX, DXG, DCR, DGG)):
                    nc.vector.scalar_tensor_tensor(
                        out=ej[:], in0=P0[:, i * 128:(i + 1) * 128],
                        scalar=0.0, in1=ident[:], op0=OP.add, op1=OP.mult,
                        accum_out=strip[:, col:col + 1])
                ej2 = tp.tile([128, 128], F32, tag="ex2")
                for i, col in enumerate((SXG1, SXG2)):
                    nc.vector.scalar_tensor_tensor(
                        out=ej2[:], in0=P1[:, i * 128:(i + 1) * 128],
                        scalar=0.0, in1=ident[:], op0=OP.add, op1=OP.mult,
                        accum_out=strip[:, col:col + 1])
                nc.vector.reduce_sum(strip[0:1, SX0:SX0 + 1], P1[0:1, 256:512],
                                     axis=AX.X)


            nc.sync.dma_start(out, strip[:])
    nc.compile()
    return nc


_NC_CACHE = None


def _get_nc():
    global _NC_CACHE
    if _NC_CACHE is None:
        _NC_CACHE = build()
    return _NC_CACHE


def _prep_core(xs, ps, gs, ts):
    """Per-core input map. xs,gs: [BC,T,D] f32; ps: [BC,T,3] f32; ts: [BC,T] i32."""
    xflat = np.zeros(((N + 2) * D,), np.float32)
    xflat[:N * D] = xs.reshape(-1)
    xw = np.lib.stride_tricks.sliding_window_view(xflat, WE)[::WR][:NCH_R * 128]
    xgrow = np.empty((NCH_R * 128, WXG), BF)
    xgrow[:, 0:WE] = xw.astype(BF)
    xgrow[:, WE:WXG] = gs.reshape(NCH_R * 128, WR).astype(BF)
    phf = np.zeros((4, N + 2), np.float32)
    phf[0:3, :N] = ps.reshape(N, 3).T
    phf[3, :N] = ts.reshape(-1)
    return {"xg": xgrow.reshape(-1), "ph": phf.astype(BF).reshape(-1)}


def _host_finish(strips, pred_robot, pred_phase):
    """strips: list of [128, NCOLS] per core. Returns f32 scalar total loss."""
    S = np.stack([s.astype(np.float64).sum(axis=0) for s in strips])  # [8, NCOLS]
    tot = S.sum(axis=0)
    svv = tot[SVV:SVV + NCH_R].sum()
    sgg = tot[DGG]
    slse = tot[SLSE]
    sxg1 = tot[SXG1]
    sxg2 = tot[SXG2]
    scnt = tot[SCNT] + tot[SCNT + 1]
    sco = tot[SCO] + tot[SCO + 1]
    sspeed = tot[SPD:SPD + 4].sum()
    sxx = tot[DXX]
    sxg = tot[DXG]
    scross = tot[DCR]
    sx0 = sum(float(s[0, SX0]) for s in strips)

    mse_sum = sxx - 2.0 * sxg + sgg

    pr = pred_robot.astype(BF).astype(np.float64)
    pp_ = pred_phase.astype(BF).astype(np.float64)

    # ---- boundary corrections (f64, tiny) ----
    svv_c = 0.0; sspeed_c = 0.0; cross_c = 0.0; edge_sum = 0.0
    cnt_c = 0.0; co_c = 0.0
    for ci in range(NCORES):
        Xb = pr[ci * BC:(ci + 1) * BC]                # [BC,T,D]
        # invalid flat vels at n = k*T-1, k=1..BC
        vbad = np.empty((BC, D))
        vbad[:BC - 1] = Xb[1:, 0] - Xb[:-1, T - 1]
        vbad[BC - 1] = -Xb[BC - 1, T - 1]             # pad-zero edge
        svv_c += (vbad ** 2).sum()
        s2b = (vbad.reshape(BC, 4, 3) ** 2).sum(-1)
        pen = np.maximum(np.sqrt(s2b) - MAX_SPEED, 0.0)
        sspeed_c += (pen ** 2).sum()
        # invalid cross products: v_{nk-1}*vbad + vbad*v_{nk+1}
        vprev = Xb[:, T - 1] - Xb[:, T - 2]           # [BC,D] last valid vel
        vnext = Xb[:, 1] - Xb[:, 0]                   # first valid vel
        cross_c += (vprev * vbad).sum()
        cross_c += (vbad[:BC - 1] * vnext[1:]).sum()
        # per-batch edge vels for the acc identity
        edge_sum += (vnext ** 2).sum() + (vprev ** 2).sum()
        # phase coherence corrections at pair t = k*T-1
        Pb = pp_[ci * BC:(ci + 1) * BC]               # [BC,T,3]
        a = Pb[:, T - 1]                              # logits at t
        b = np.zeros_like(a)
        b[:BC - 1] = Pb[1:, 0]                        # logits at t+1 (pad zero last)
        ma_ = a.max(-1); mb_ = b.max(-1)
        ua = (a[:, 1] == ma_) + 2.0 * (a[:, 2] == ma_)
        ub = (b[:, 1] == mb_) + 2.0 * (b[:, 2] == mb_)
        dd = ub - ua
        f = (dd - 1.0) * dd
        mask = np.minimum(f, 1.0)
        cnt_c += mask.sum()
        co_c += (mask * mb_ ** 2).sum()

    svv_t = svv - svv_c
    cross_t = scross - cross_c
    sspeed_t = sspeed - sspeed_c
    acc_sum = 2.0 * svv_t - edge_sum - 2.0 * cross_t
    cnt_t = scnt - cnt_c
    co_t = sco - co_c

    robot_loss = mse_sum / (B * T * D)
    xgt = sx0 + sxg1 + sxg2
    phase_loss = (slse - xgt) / (B * T)
    coherence = (co_t / max(cnt_t, 1.0)) if cnt_t > 0 else 0.0
    speed_loss = 5.0 * sspeed_t / (B * (T - 1) * 4)
    vel_loss = svv_t / (B * (T - 1) * D)
    acc_loss = acc_sum / (B * (T - 2) * D)
    total = (robot_loss + phase_loss + 10.0 * coherence + speed_loss
             + 0.05 * vel_loss + 0.01 * acc_loss)
    return np.asarray(total, dtype=np.float32)


def kernel(pred_robot, pred_phase, gt_robot, gt_phase):
    nc = _get_nc()
    in_maps = []
    for c in range(NCORES):
        sl = slice(c * BC, (c + 1) * BC)
        in_maps.append(_prep_core(pred_robot[sl], pred_phase[sl],
                                  gt_robot[sl], gt_phase[sl]))
    res = bass_utils.run_bass_kernel_spmd(nc, in_maps, core_ids=list(range(NCORES)))
    strips = [res.results[c]["partials"] for c in range(NCORES)]
    return _host_finish(strips, pred_robot, pred_phase)
